# revision 42
# baseline (speedup 1.0000x reference)
"""Multi-head self-attention TRN2 Bass kernel (v2).

Sharding: 8 cores = 4 batches x 2 head-halves. Core (b, half) computes
batch b with 8 heads (half*8 .. half*8+8), producing a [CAP, 1024]
partial of the output projection; the host sums the 2 halves per batch,
scatters compacted rows back to their original positions, and adds the
constant bv @ Wc.T row (softmax weights sum to 1, so the V-bias
contributes a constant vector).

Host-side compaction: only unmasked tokens (mask==0) take part in
attention -- masked queries produce zero rows and masked keys are
excluded. Tokens are compacted per batch and padded to CAP=1152
(valid counts are ~1004-1036), cutting attention work to (1152/2048)^2
and projection work to 1152/2048.

All matmuls run in fp16 (1 cycle/row on the PE at full clock, fast
weight loads); fp32r was 2-4x slower per row because it self-loads
weights and kept the PE at the cold 1.2 GHz p-state. Dots for the two
heads of a pair run concurrently via PE row-tiling (tile_position
(0,0)/(64,0), contraction 64 each).

Attention runs as a software pipeline over (pair, query-chunk) steps:
dots for both heads of a pair are adjacent row-tiled matmuls (rows
0-63 / 64-127) sharing one [128, 2, 512] PSUM tile, so one exp
activation (p = exp(dots/8 - 1)) covers both heads; the previous
step's AV matmuls interleave per key-tile so the PE always has ready
work while exp paces the dots. Padding keys are masked by zeroing
their V' rows and ones-column (inv), padded queries are discarded by
the host scatter. The diagonal is zeroed by a DVE multiply with (1-I)
on the 128-column window containing it. The softmax denominator comes
from a 65th all-ones V' column; denominators from four (head, chunk)
steps are batched into one [4-row, 512] DVE reciprocal (rows at
partitions 0/32/64/96 so the rep broadcast matmul can use them as
1-row moving operands), and the rep-broadcast + normalize trail two
steps behind the AV so the reciprocal latency is fully hidden.
"""

import os
import numpy as np
import ml_dtypes

import concourse.bacc as bacc
import concourse.mybir as mybir
from concourse.tile import TileContext
from concourse.bass_utils import run_bass_kernel_spmd

B, S, H, NH, HD = 4, 2048, 1024, 16, 64
NCORES = 8
HPC = 8                   # heads per core
PD = HPC * HD             # per-core projection dim = 512
CAP = 1152                # compacted token capacity per batch
KT = CAP // 128           # 9 key tiles
FT = H // 128             # 8 feature k-tiles
QCH = [(0, 512), (512, 512), (1024, 128)]   # query chunks
F32 = mybir.dt.float32
F16 = mybir.dt.float16

LAST_RESULTS = None       # BassKernelResults from the most recent run


def build_bass(cap=CAP, fused_pairs=3):
    """fused_pairs: number of leading kt-pairs whose keys are known
    all-valid (bias -1 immediate, one exp instr per 2 key tiles).
    The remaining kt get per-kt exp instrs with a per-partition bias AP
    that encodes padding. fused_pairs=3 requires nb >= 896."""
    kt = cap // 128
    qch_full = [(o, n) for (o, n) in [(i * 512, min(512, cap - i * 512))
                                      for i in range((cap + 511) // 512)]
                if n > 0]
    qch = list(qch_full)
    if cap == CAP:
        # attention stages only need QUERY columns < nb (max valid count
        # is ~1036); trim the last chunk 128 -> 32 (covers nb <= 1056,
        # checked at dispatch). Projections must still cover all of cap:
        # the dots stationary reads all KEY columns.
        qch[-1] = (qch[-1][0], 32)
    nc = bacc.Bacc()
    xcT = nc.dram_tensor("xcT", [H, cap], F16, kind="ExternalInput")
    wq = nc.dram_tensor("wq", [H, PD], F16, kind="ExternalInput")
    wk = nc.dram_tensor("wk", [H, PD], F16, kind="ExternalInput")
    wv = nc.dram_tensor("wv", [H, PD], F16, kind="ExternalInput")
    wc = nc.dram_tensor("wc", [PD, H], F16, kind="ExternalInput")
    bqk = nc.dram_tensor("bqk", [128, 8], F32, kind="ExternalInput")
    padb = nc.dram_tensor("padb", [128, kt], F32, kind="ExternalInput")
    eyebar = nc.dram_tensor("eyebar", [128, 128], F16, kind="ExternalInput")
    inv16 = nc.dram_tensor("inv16", [128, kt], F32, kind="ExternalInput")
    ones64 = nc.dram_tensor("ones64", [128, 64], F16, kind="ExternalInput")
    outp = nc.dram_tensor("out", [cap, H], F16, kind="ExternalOutput")

    EXP = mybir.ActivationFunctionType.Exp
    DR = None  # no DoubleRow (fp16 path)

    # exp schedule: padding keys are masked via zeroed V'/ones columns
    # (not exp bias), so every exp instr uses the constant -1 bias and
    # key tiles fuse uniformly into pairs
    sched = [((2 * i, 2 * i + 1), None) for i in range(kt // 2)]
    if kt % 2:
        sched.append(((kt - 1,), None))

    with TileContext(nc) as tc, \
         tc.tile_pool(name="consts", bufs=1) as cpool, \
         tc.tile_pool(name="work", bufs=1) as wpool, \
         tc.tile_pool(name="ptp", bufs=4) as ppool, \
         tc.tile_pool(name="small", bufs=3) as spool, \
         tc.tile_pool(name="osb", bufs=2) as opool, \
         tc.tile_pool(name="psum", bufs=1, space="PSUM") as pspool:

        # ---- constants / weights ----
        # xcT and the q/k weights stream in first so the first projection
        # matmuls start as soon as possible; wv/wc and constants follow
        xc_sb = wpool.tile([128, FT, cap], F16, name="xcsb")
        w_sb = {}
        for name, t in (("q", wq), ("k", wk), ("v", wv)):
            w_sb[name] = cpool.tile([128, FT, PD], F16, name=f"w{name}sb")
        for ft in range(FT):
            nc.sync.dma_start(out=xc_sb[:, ft, :],
                              in_=xcT[ft * 128:(ft + 1) * 128, :])
            for name, t in (("q", wq), ("k", wk)):
                nc.sync.dma_start(out=w_sb[name][:, ft, :],
                                  in_=t[ft * 128:(ft + 1) * 128, :])
        bqk_sb = cpool.tile([128, 8], F32, name="bqksb")
        nc.sync.dma_start(out=bqk_sb[:, :], in_=bqk[:, :])
        padb_sb = cpool.tile([128, kt], F32, name="padbsb")
        nc.sync.dma_start(out=padb_sb[:, :], in_=padb[:, :])
        eye_sb = cpool.tile([128, 128], F16, name="eyesb")
        nc.sync.dma_start(out=eye_sb[:, :], in_=eyebar[:, :])
        inv_sb = cpool.tile([128, kt], F32, name="invsb")
        nc.sync.dma_start(out=inv_sb[:, :], in_=inv16[:, :])
        ones_sb = cpool.tile([128, 64], F16, name="onessb")
        nc.sync.dma_start(out=ones_sb[:, :], in_=ones64[:, :])
        for ft in range(FT):
            nc.sync.dma_start(out=w_sb["v"][:, ft, :],
                              in_=wv[ft * 128:(ft + 1) * 128, :])
        wc_sb = cpool.tile([128, 4, H], F16, name="wcsb")
        for g in range(4):
            nc.sync.dma_start(out=wc_sb[:, g, :],
                              in_=wc[g * 128:(g + 1) * 128, :])

        # ---- q/k projections -> qkT [128, 4, cap] f16 ----
        # group-major order: head-pair p only needs group g=p, so pair 0's
        # attention can start after the first quarter of the projections
        # and overlap the rest
        qkT = {w: wpool.tile([128, 4, cap], F16, name=f"{w}T")
               for w in "qk"}

        def emit_qk_group(g):
            for wi, w in enumerate("qk"):
                for qo, qn in qch_full:
                    pp = pspool.tile([128, 512], F32, tag="mm", bufs=2)
                    for ft in range(FT):
                        nc.tensor.matmul(
                            pp[:, 0:qn],
                            w_sb[w][:, ft, g * 128:(g + 1) * 128],
                            xc_sb[:, ft, qo:qo + qn],
                            start=(ft == 0), stop=(ft == FT - 1))
                    nc.vector.tensor_scalar_add(
                        qkT[w][:, g, qo:qo + qn], pp[:, 0:qn],
                        bqk_sb[:, 4 * wi + g:4 * wi + g + 1])

        # HAM warm-up: dummy matmuls during the input-DMA wait ramp the
        # PE clock to 2.4 GHz before the first projection
        wu = cpool.tile([128, 512], F16, name="wu")
        nc.vector.memset(wu[:, :], 0.0)
        for _ in range(28):
            wps = pspool.tile([128, 512], F32, tag="mm", bufs=2, name="wps")
            nc.tensor.matmul(wps[:, :], wu[:, 0:128], wu[:, :],
                             start=True, stop=True)

        emit_qk_group(0)

        # ---- v projection (token-major) -> vp [128, kt, 8, 65] f16 ----
        # vp[key, kt, h, m] = V[key, h*64+m]; vp[key, kt, h, 64] = 1 (denom)
        # Emitted from the driver AFTER attention step 0 so step 0's exp
        # burst overlaps this PE-only block; step 1's AV (first vp reader)
        # still follows it in the queue.
        vp = wpool.tile([128, kt, HPC, 65], F16, name="vp")

        def emit_vproj():
            for tt in range(kt):
                pv = pspool.tile([128, 512], F32, tag="mm", bufs=2,
                                 name="pv")
                for ft in range(FT):
                    nc.tensor.matmul(
                        pv[:, :],
                        xc_sb[:, ft, tt * 128:(tt + 1) * 128],
                        w_sb["v"][:, ft, :],
                        start=(ft == 0), stop=(ft == FT - 1))
                # strided copy with padding-key mask: [128, 8, 64] from pv
                nc.vector.tensor_scalar_mul(
                    vp[:, tt, :, 0:64],
                    pv.rearrange("p (h d) -> p h d", d=64),
                    inv_sb[:, tt:tt + 1])
                nc.vector.tensor_copy(
                    vp[:, tt, :, 64:65],
                    inv_sb[:, tt:tt + 1].to_broadcast((128, HPC, 1)))

        # ---- attention, 3-stage software pipeline over (pair, qchunk) ----
        # stage 1: dots + exp + diag -> ptp tiles
        # stage 2 (one step behind): AV accumulation + reciprocal issue
        # stage 3 (two steps behind): rep broadcast + normalize
        # This keeps the PE fed: exp(i) overlaps AV(i-1) on the PE, and the
        # slow [1,N] reciprocal gets a full step of slack before rep reads it.
        onorm = wpool.tile([128, 4, cap], F16, name="onorm")
        att_end = qch[-1][0] + qch[-1][1]
        if att_end < cap:
            nc.vector.memset(onorm[:, :, att_end:cap], 0.0)
        avq = []        # (ptp pair, p, qo, qn) awaiting AV
        deferred = []   # (osb, rcref, row, p, rb, qo, qn) awaiting normalize
        den_st = {"tile": None, "row": 0, "rc": None}
        normed = {}     # qo -> count of normalized (pair, head) slices
        tt_of = {o: (o // 128, (o + n) // 128) for o, n in qch_full}

        def emit_outproj(qo_d):
            t0, t1 = tt_of[qo_d]
            for tt in range(t0, t1):
                osb = opool.tile([128, H], F16, tag="osb", name="osbout")
                for oc in range(2):
                    op = pspool.tile([128, 512], F32, tag="mm", bufs=2,
                                     name="op")
                    for g in range(4):
                        nc.tensor.matmul(
                            op[:, :],
                            onorm[:, g, tt * 128:(tt + 1) * 128],
                            wc_sb[:, g, oc * 512:(oc + 1) * 512],
                            start=(g == 0), stop=(g == 3))
                    if (tt + oc) % 2 == 0:
                        nc.vector.tensor_copy(
                            osb[:, oc * 512:(oc + 1) * 512], op[:, :])
                    else:
                        nc.scalar.copy(
                            osb[:, oc * 512:(oc + 1) * 512], op[:, :])
                nc.sync.dma_start(
                    out=outp[tt * 128:(tt + 1) * 128, :], in_=osb[:, :])

        def flush_norm(count):
            for _ in range(count):
                if not deferred or deferred[0][1]["rc"] is None:
                    return
                osb_d, rcref, row, p_d, rb_d, qo_d, qn_d = deferred.pop(0)
                rc16 = rcref["rc"]
                rep = pspool.tile([64, 512], F32, tag="mm", bufs=2,
                                  name="rep")
                nc.tensor.matmul(rep[:, 0:qn_d],
                                 ones_sb[32 * row:32 * row + 1, :],
                                 rc16[32 * row:32 * row + 1, 0:qn_d],
                                 start=True, stop=True,
                                 tile_position=(32 * row, 0))
                nc.vector.tensor_mul(
                    onorm[rb_d:rb_d + 64, p_d, qo_d:qo_d + qn_d],
                    osb_d[:, 0:qn_d], rep[:, 0:qn_d])
                normed[qo_d] = normed.get(qo_d, 0) + 1
                if normed[qo_d] == 8:
                    emit_outproj(qo_d)

        def finish_av(avs2, p_a, qo_a, qn_a):
            # drain avs to SBUF (frees PSUM), batch reciprocals
            for h01 in range(2):
                avs = avs2[h01]
                rb = h01 * 64
                osb = spool.tile([64, 512], F16, tag="osb", bufs=6,
                                 name="oun")
                nc.vector.tensor_copy(osb[:, 0:qn_a], avs[0:64, 0:qn_a])
                if den_st["tile"] is None:
                    den_st["tile"] = spool.tile([128, 512], F32,
                                                tag="den4", bufs=2,
                                                name="den4")
                    nc.vector.memset(den_st["tile"][:, :], 1.0)
                    den_st["row"] = 0
                    den_st["rc"] = {"rc": None}
                d4, row = den_st["tile"], den_st["row"]
                nc.vector.tensor_copy(d4[32 * row:32 * row + 1, 0:qn_a],
                                      avs[64:65, 0:qn_a])
                deferred.append((osb, den_st["rc"], row, p_a, rb,
                                 qo_a, qn_a))
                den_st["row"] += 1
                if den_st["row"] == 4:
                    # one reciprocal covers 4 denominators (4 DVE lanes
                    # run in parallel; [1,N] and [4,N] cost the same)
                    rc32 = spool.tile([128, 512], F32, tag="rc32",
                                      bufs=2, name="rc32")
                    with nc.allow_low_precision(
                            reason="1/den in fp16: 0.05% rel, den>=13"):
                        nc.vector.reciprocal(rc32[0:97, :], d4[0:97, :])
                    rc16 = spool.tile([128, 512], F16, tag="rc16", bufs=2,
                                      name="rc16")
                    nc.vector.tensor_copy(rc16[0:97, :], rc32[0:97, :])
                    den_st["rc"]["rc"] = rc16
                    den_st["tile"] = None

        def do_step(p, qo, qn):
            if True:
                flush_norm(2)
                prev = avq.pop(0) if avq else None
                if prev:
                    ptp_a, p_a, qo_a, qn_a = prev
                    avs2 = [pspool.tile([65, 512], F32, tag="avs", bufs=2,
                                        name="avs") for _ in range(2)]
                ptp = ppool.tile([128, kt, 2, 512], F16, tag="ptp",
                                 bufs=(4 if cap == CAP else 2),
                                 name=f"ptp{p}{qo}")
                for k in range(kt):
                    # both heads' dots adjacent at row groups 0/64 -> the
                    # PE runs them concurrently; one exp covers both.
                    # Previous step's AV matmuls are interleaved per-k so
                    # the PE has ready work while exp paces the dots.
                    dp = pspool.tile([128, 2, 512], F32, tag="dp", bufs=2)
                    for h01 in range(2):
                        rb = h01 * 64
                        nc.tensor.matmul(
                            dp[:, h01, 0:qn],
                            qkT["k"][rb:rb + 64, p,
                                     k * 128:(k + 1) * 128],
                            qkT["q"][rb:rb + 64, p, qo:qo + qn],
                            start=True, stop=True,
                            tile_position=(rb, 0))
                    nc.scalar.activation(
                        ptp[:, k, :, 0:qn], dp[:, :, 0:qn],
                        EXP, scale=0.125, bias=padb_sb[:, 0:1])
                    # diagonal zeroing when this kt's window is in-chunk
                    if qo <= k * 128 < qo + qn:
                        off = k * 128 - qo
                        w = min(qn - off, 128)
                        for h01 in range(2):
                            nc.vector.tensor_mul(
                                ptp[:, k, h01, off:off + w],
                                ptp[:, k, h01, off:off + w],
                                eye_sb[:, 0:w])
                    if prev:
                        for h01 in range(2):
                            nc.tensor.matmul(
                                avs2[h01][:, 0:qn_a],
                                vp[:, k, 2 * p_a + h01, :],
                                ptp_a[:, k, h01, 0:qn_a],
                                start=(k == 0), stop=(k == kt - 1),
                                perf_mode=DR)
                if prev:
                    finish_av(avs2, p_a, qo_a, qn_a)
                avq.append((ptp, p, qo, qn))

        # interleave attention steps with the remaining projection groups:
        # steps 0-1 need only groups 0-1, so the PE FIFO alternates proj
        # bursts with exp-paced attention instead of serializing them
        steps = [(p, qo, qn) for (qo, qn) in qch for p in range(4)]
        do_step(*steps[0])
        emit_vproj()
        emit_qk_group(1)
        do_step(*steps[1])
        emit_qk_group(2)
        do_step(*steps[2])
        emit_qk_group(3)
        for st in steps[3:]:
            do_step(*st)

        def finish_partial_rc():
            if den_st["tile"] is not None and den_st["row"] > 0:
                d4 = den_st["tile"]
                rc32 = spool.tile([128, 512], F32, tag="rc32",
                                  bufs=2, name="rc32")
                with nc.allow_low_precision(
                        reason="1/den in fp16: 0.05% rel, den>=13"):
                    nc.vector.reciprocal(rc32[0:97, :], d4[0:97, :])
                rc16 = spool.tile([128, 512], F16, bufs=2, tag="rc16",
                                  name="rc16")
                nc.vector.tensor_copy(rc16[0:97, :], rc32[0:97, :])
                den_st["rc"]["rc"] = rc16
                den_st["tile"] = None

        # tail: AV + normalize for the final step
        ptp_a, p_a, qo_a, qn_a = avq.pop(0)
        avs2 = [pspool.tile([65, 512], F32, tag="avs", bufs=2, name="avs")
                for _ in range(2)]
        for k in range(kt):
            for h01 in range(2):
                nc.tensor.matmul(
                    avs2[h01][:, 0:qn_a],
                    vp[:, k, 2 * p_a + h01, :],
                    ptp_a[:, k, h01, 0:qn_a],
                    start=(k == 0), stop=(k == kt - 1),
                    perf_mode=DR)
        finish_av(avs2, p_a, qo_a, qn_a)
        finish_partial_rc()
        flush_norm(len(deferred))

        # defensive: emit any output chunk not already flushed out
        for qo_d, _ in qch:
            if normed.get(qo_d, 0) != 8 and qo_d in tt_of:
                raise AssertionError(f"chunk {qo_d} normed "
                                     f"{normed.get(qo_d, 0)} != 8")
    nc.finalize()
    return nc


_NC_CACHE = {}


def _get_nc(cap, fused_pairs):
    key = (cap, fused_pairs)
    if key not in _NC_CACHE:
        _NC_CACHE[key] = build_bass(cap, fused_pairs)
    return _NC_CACHE[key]


def kernel(encoder_outputs, mask, Wq, bq, Wk, bk, Wv, bv, Wc):
    global LAST_RESULTS
    x = np.asarray(encoder_outputs, dtype=np.float32)
    mask = np.asarray(mask)
    f16 = np.float16
    Wqh, Wkh, Wvh = (np.asarray(w, np.float32) for w in (Wq, Wk, Wv))
    Wch = np.asarray(Wc, np.float32)

    idxs = [np.where(mask[b] == 0)[0] for b in range(B)]
    nbs = [len(i) for i in idxs]
    if max(nbs) <= 1024 + 32 and min(nbs) >= 896:
        cap, fused = CAP, 3    # attention q-range trimmed to 1056
    else:
        cap, fused = S, 0          # generic fallback: no compaction gain
    kt = cap // 128

    eyebar = (1.0 - np.eye(128, dtype=np.float32)).astype(f16)
    onesv = np.ones((128, 64), dtype=f16)

    in_maps = []
    for c in range(NCORES):
        b, half = c // 2, c % 2
        hsl = slice(half * PD, (half + 1) * PD)
        idx, nb = idxs[b], nbs[b]
        xc = np.zeros((cap, H), np.float32)
        xc[:nb] = x[b, idx[:nb]]
        xcT = np.ascontiguousarray(xc.T).astype(f16)
        padb = np.full(cap, -1.0, dtype=np.float32)
        inv = (np.arange(cap) < nb).astype(np.float32)
        bqk = np.stack([np.asarray(bq, np.float32)[hsl].reshape(4, 128),
                        np.asarray(bk, np.float32)[hsl].reshape(4, 128)]
                       ).reshape(8, 128).T.copy()
        in_maps.append({
            "xcT": xcT,
            "wq": np.ascontiguousarray(Wqh[hsl, :].T).astype(f16),
            "wk": np.ascontiguousarray(Wkh[hsl, :].T).astype(f16),
            "wv": np.ascontiguousarray(Wvh[hsl, :].T).astype(f16),
            "wc": np.ascontiguousarray(Wch[:, hsl].T).astype(f16),
            "bqk": bqk,
            "padb": np.ascontiguousarray(padb.reshape(kt, 128).T),
            "eyebar": eyebar,
            "inv16": np.ascontiguousarray(
                inv.reshape(kt, 128).T.astype(np.float32)),
            "ones64": onesv,
        })

    res = run_bass_kernel_spmd(
        _get_nc(cap, fused), in_maps, list(range(NCORES)),
        trace=bool(os.environ.get("BASS_TRACE")))
    LAST_RESULTS = res

    bvWc = (np.asarray(bv, np.float64) @ np.asarray(Wc, np.float64).T
            ).astype(np.float32)
    out = np.zeros((B, S, H), dtype=np.float32)
    for b in range(B):
        nb = nbs[b]
        part = (res.results[2 * b]["out"][:nb].astype(np.float32)
                + res.results[2 * b + 1]["out"][:nb].astype(np.float32))
        out[b, idxs[b][:nb]] = part + bvWc[None, :]
    return out


# revision 43
# speedup vs baseline: 1.0250x; 1.0250x over previous
"""Multi-head self-attention TRN2 Bass kernel (v2).

Sharding: 8 cores = 4 batches x 2 head-halves. Core (b, half) computes
batch b with 8 heads (half*8 .. half*8+8), producing a [CAP, 1024]
partial of the output projection; the host sums the 2 halves per batch,
scatters compacted rows back to their original positions, and adds the
constant bv @ Wc.T row (softmax weights sum to 1, so the V-bias
contributes a constant vector).

Host-side compaction: only unmasked tokens (mask==0) take part in
attention -- masked queries produce zero rows and masked keys are
excluded. Tokens are compacted per batch and padded to CAP=1152
(valid counts are ~1004-1036), cutting attention work to (1152/2048)^2
and projection work to 1152/2048.

All matmuls run in fp16 (1 cycle/row on the PE at full clock, fast
weight loads); fp32r was 2-4x slower per row because it self-loads
weights and kept the PE at the cold 1.2 GHz p-state. Dots for the two
heads of a pair run concurrently via PE row-tiling (tile_position
(0,0)/(64,0), contraction 64 each).

Attention runs as a software pipeline over (pair, query-chunk) steps:
dots for both heads of a pair are adjacent row-tiled matmuls (rows
0-63 / 64-127) sharing one [128, 2, 512] PSUM tile, so one exp
activation (p = exp(dots/8 - 1)) covers both heads; the previous
step's AV matmuls interleave per key-tile so the PE always has ready
work while exp paces the dots. Padding keys are masked by zeroing
their V' rows and ones-column (inv), padded queries are discarded by
the host scatter. The diagonal is zeroed by a DVE multiply with (1-I)
on the 128-column window containing it. The softmax denominator comes
from a 65th all-ones V' column; denominators from four (head, chunk)
steps are batched into one [4-row, 512] DVE reciprocal (rows at
partitions 0/32/64/96 so the rep broadcast matmul can use them as
1-row moving operands), and the rep-broadcast + normalize trail two
steps behind the AV so the reciprocal latency is fully hidden.
"""

import os
import numpy as np
import ml_dtypes

import concourse.bacc as bacc
import concourse.mybir as mybir
from concourse.tile import TileContext
from concourse.bass_utils import run_bass_kernel_spmd

B, S, H, NH, HD = 4, 2048, 1024, 16, 64
NCORES = 8
HPC = 8                   # heads per core
PD = HPC * HD             # per-core projection dim = 512
CAP = 1152                # compacted token capacity per batch
KT = CAP // 128           # 9 key tiles
FT = H // 128             # 8 feature k-tiles
QCH = [(0, 512), (512, 512), (1024, 128)]   # query chunks
F32 = mybir.dt.float32
F16 = mybir.dt.float16

LAST_RESULTS = None       # BassKernelResults from the most recent run


def build_bass(cap=CAP, fused_pairs=3):
    """fused_pairs: number of leading kt-pairs whose keys are known
    all-valid (bias -1 immediate, one exp instr per 2 key tiles).
    The remaining kt get per-kt exp instrs with a per-partition bias AP
    that encodes padding. fused_pairs=3 requires nb >= 896."""
    kt = cap // 128
    qch_full = [(o, n) for (o, n) in [(i * 512, min(512, cap - i * 512))
                                      for i in range((cap + 511) // 512)]
                if n > 0]
    qch = list(qch_full)
    if cap == CAP:
        # attention stages only need QUERY columns < nb (max valid count
        # is ~1036); trim the last chunk 128 -> 32 (covers nb <= 1056,
        # checked at dispatch). Projections must still cover all of cap:
        # the dots stationary reads all KEY columns.
        qch[-1] = (qch[-1][0], 32)
    nc = bacc.Bacc()
    xcT = nc.dram_tensor("xcT", [H, cap], F16, kind="ExternalInput")
    wq = nc.dram_tensor("wq", [H, PD], F16, kind="ExternalInput")
    wk = nc.dram_tensor("wk", [H, PD], F16, kind="ExternalInput")
    wv = nc.dram_tensor("wv", [H, PD], F16, kind="ExternalInput")
    wc = nc.dram_tensor("wc", [PD, H], F16, kind="ExternalInput")
    bqk = nc.dram_tensor("bqk", [128, 8], F32, kind="ExternalInput")
    padb = nc.dram_tensor("padb", [128, kt], F32, kind="ExternalInput")
    eyebar = nc.dram_tensor("eyebar", [128, 128], F16, kind="ExternalInput")
    inv16 = nc.dram_tensor("inv16", [128, kt], F32, kind="ExternalInput")
    ones64 = nc.dram_tensor("ones64", [128, 64], F16, kind="ExternalInput")
    outp = nc.dram_tensor("out", [cap, H], F16, kind="ExternalOutput")

    EXP = mybir.ActivationFunctionType.Exp
    DR = None  # no DoubleRow (fp16 path)

    # exp schedule: padding keys are masked via zeroed V'/ones columns
    # (not exp bias), so every exp instr uses the constant -1 bias and
    # key tiles fuse uniformly into pairs
    sched = [((2 * i, 2 * i + 1), None) for i in range(kt // 2)]
    if kt % 2:
        sched.append(((kt - 1,), None))

    with TileContext(nc) as tc, \
         tc.tile_pool(name="consts", bufs=1) as cpool, \
         tc.tile_pool(name="work", bufs=1) as wpool, \
         tc.tile_pool(name="ptp", bufs=4) as ppool, \
         tc.tile_pool(name="small", bufs=3) as spool, \
         tc.tile_pool(name="osb", bufs=2) as opool, \
         tc.tile_pool(name="psum", bufs=1, space="PSUM") as pspool:

        # ---- constants / weights ----
        # xcT and the q/k weights stream in first so the first projection
        # matmuls start as soon as possible; wv/wc and constants follow
        xc_sb = wpool.tile([128, FT, cap], F16, name="xcsb")
        w_sb = {}
        for name, t in (("q", wq), ("k", wk), ("v", wv)):
            w_sb[name] = cpool.tile([128, FT, PD], F16, name=f"w{name}sb")
        for ft in range(FT):
            nc.sync.dma_start(out=xc_sb[:, ft, :],
                              in_=xcT[ft * 128:(ft + 1) * 128, :])
            for name, t in (("q", wq), ("k", wk)):
                nc.sync.dma_start(out=w_sb[name][:, ft, :],
                                  in_=t[ft * 128:(ft + 1) * 128, :])
        bqk_sb = cpool.tile([128, 8], F32, name="bqksb")
        nc.sync.dma_start(out=bqk_sb[:, :], in_=bqk[:, :])
        padb_sb = cpool.tile([128, kt], F32, name="padbsb")
        nc.sync.dma_start(out=padb_sb[:, :], in_=padb[:, :])
        eye_sb = cpool.tile([128, 128], F16, name="eyesb")
        nc.sync.dma_start(out=eye_sb[:, :], in_=eyebar[:, :])
        inv_sb = cpool.tile([128, kt], F32, name="invsb")
        nc.sync.dma_start(out=inv_sb[:, :], in_=inv16[:, :])
        ones_sb = cpool.tile([128, 64], F16, name="onessb")
        nc.sync.dma_start(out=ones_sb[:, :], in_=ones64[:, :])
        for ft in range(FT):
            nc.sync.dma_start(out=w_sb["v"][:, ft, :],
                              in_=wv[ft * 128:(ft + 1) * 128, :])
        wc_sb = cpool.tile([128, 4, H], F16, name="wcsb")
        for g in range(4):
            nc.sync.dma_start(out=wc_sb[:, g, :],
                              in_=wc[g * 128:(g + 1) * 128, :])

        # ---- q/k projections -> qkT [128, 4, cap] f16 ----
        # group-major order: head-pair p only needs group g=p, so pair 0's
        # attention can start after the first quarter of the projections
        # and overlap the rest
        qkT = {w: wpool.tile([128, 4, cap], F16, name=f"{w}T")
               for w in "qk"}

        def emit_qk_group(g):
            for wi, w in enumerate("qk"):
                for qo, qn in qch_full:
                    pp = pspool.tile([128, 512], F32, tag="mm", bufs=2)
                    for ft in range(FT):
                        nc.tensor.matmul(
                            pp[:, 0:qn],
                            w_sb[w][:, ft, g * 128:(g + 1) * 128],
                            xc_sb[:, ft, qo:qo + qn],
                            start=(ft == 0), stop=(ft == FT - 1))
                    nc.vector.tensor_scalar_add(
                        qkT[w][:, g, qo:qo + qn], pp[:, 0:qn],
                        bqk_sb[:, 4 * wi + g:4 * wi + g + 1])

        # HAM warm-up: dummy matmuls during the input-DMA wait ramp the
        # PE clock to 2.4 GHz before the first projection
        wu = cpool.tile([128, 512], F16, name="wu")
        nc.vector.memset(wu[:, :], 0.0)
        for _ in range(28):
            wps = pspool.tile([128, 512], F32, tag="mm", bufs=2, name="wps")
            nc.tensor.matmul(wps[:, :], wu[:, 0:128], wu[:, :],
                             start=True, stop=True)

        emit_qk_group(0)

        # ---- v projection (token-major) -> vp [128, kt, 8, 65] f16 ----
        # vp[key, kt, h, m] = V[key, h*64+m]; vp[key, kt, h, 64] = 1 (denom)
        # Emitted from the driver AFTER attention step 0 so step 0's exp
        # burst overlaps this PE-only block; step 1's AV (first vp reader)
        # still follows it in the queue.
        vp = wpool.tile([128, kt, HPC, 65], F16, name="vp")

        def emit_vproj(tts):
            for tt in tts:
                pv = pspool.tile([128, 512], F32, tag="mm", bufs=2,
                                 name="pv")
                for ft in range(FT):
                    nc.tensor.matmul(
                        pv[:, :],
                        xc_sb[:, ft, tt * 128:(tt + 1) * 128],
                        w_sb["v"][:, ft, :],
                        start=(ft == 0), stop=(ft == FT - 1))
                # strided copy with padding-key mask: [128, 8, 64] from pv
                nc.vector.tensor_scalar_mul(
                    vp[:, tt, :, 0:64],
                    pv.rearrange("p (h d) -> p h d", d=64),
                    inv_sb[:, tt:tt + 1])
                nc.vector.tensor_copy(
                    vp[:, tt, :, 64:65],
                    inv_sb[:, tt:tt + 1].to_broadcast((128, HPC, 1)))

        # ---- attention, 3-stage software pipeline over (pair, qchunk) ----
        # stage 1: dots + exp + diag -> ptp tiles
        # stage 2 (one step behind): AV accumulation + reciprocal issue
        # stage 3 (two steps behind): rep broadcast + normalize
        # This keeps the PE fed: exp(i) overlaps AV(i-1) on the PE, and the
        # slow [1,N] reciprocal gets a full step of slack before rep reads it.
        onorm = wpool.tile([128, 4, cap], F16, name="onorm")
        att_end = qch[-1][0] + qch[-1][1]
        if att_end < cap:
            nc.vector.memset(onorm[:, :, att_end:cap], 0.0)
        avq = []        # (ptp pair, p, qo, qn) awaiting AV
        deferred = []   # (osb, rcref, row, p, rb, qo, qn) awaiting normalize
        den_st = {"tile": None, "row": 0, "rc": None}
        normed = {}     # qo -> count of normalized (pair, head) slices
        tt_of = {o: (o // 128, (o + n) // 128) for o, n in qch_full}

        def emit_outproj(qo_d):
            t0, t1 = tt_of[qo_d]
            for tt in range(t0, t1):
                osb = opool.tile([128, H], F16, tag="osb", name="osbout")
                for oc in range(2):
                    op = pspool.tile([128, 512], F32, tag="mm", bufs=2,
                                     name="op")
                    for g in range(4):
                        nc.tensor.matmul(
                            op[:, :],
                            onorm[:, g, tt * 128:(tt + 1) * 128],
                            wc_sb[:, g, oc * 512:(oc + 1) * 512],
                            start=(g == 0), stop=(g == 3))
                    if (tt + oc) % 2 == 0:
                        nc.vector.tensor_copy(
                            osb[:, oc * 512:(oc + 1) * 512], op[:, :])
                    else:
                        nc.scalar.copy(
                            osb[:, oc * 512:(oc + 1) * 512], op[:, :])
                nc.sync.dma_start(
                    out=outp[tt * 128:(tt + 1) * 128, :], in_=osb[:, :])

        def flush_norm(count):
            for _ in range(count):
                if not deferred or deferred[0][1]["rc"] is None:
                    return
                osb_d, rcref, row, p_d, rb_d, qo_d, qn_d = deferred.pop(0)
                rc16 = rcref["rc"]
                rep = pspool.tile([64, 512], F32, tag="mm", bufs=2,
                                  name="rep")
                nc.tensor.matmul(rep[:, 0:qn_d],
                                 ones_sb[32 * row:32 * row + 1, :],
                                 rc16[32 * row:32 * row + 1, 0:qn_d],
                                 start=True, stop=True,
                                 tile_position=(32 * row, 0))
                nc.vector.tensor_mul(
                    onorm[rb_d:rb_d + 64, p_d, qo_d:qo_d + qn_d],
                    osb_d[:, 0:qn_d], rep[:, 0:qn_d])
                normed[qo_d] = normed.get(qo_d, 0) + 1
                if normed[qo_d] == 8:
                    emit_outproj(qo_d)

        def finish_av(avs2, p_a, qo_a, qn_a):
            # drain avs to SBUF (frees PSUM), batch reciprocals
            for h01 in range(2):
                avs = avs2[h01]
                rb = h01 * 64
                osb = spool.tile([64, 512], F16, tag="osb", bufs=6,
                                 name="oun")
                nc.vector.tensor_copy(osb[:, 0:qn_a], avs[0:64, 0:qn_a])
                if den_st["tile"] is None:
                    den_st["tile"] = spool.tile([128, 512], F32,
                                                tag="den4", bufs=2,
                                                name="den4")
                    nc.vector.memset(den_st["tile"][:, :], 1.0)
                    den_st["row"] = 0
                    den_st["rc"] = {"rc": None}
                d4, row = den_st["tile"], den_st["row"]
                nc.vector.tensor_copy(d4[32 * row:32 * row + 1, 0:qn_a],
                                      avs[64:65, 0:qn_a])
                deferred.append((osb, den_st["rc"], row, p_a, rb,
                                 qo_a, qn_a))
                den_st["row"] += 1
                if den_st["row"] == 4:
                    # one reciprocal covers 4 denominators (4 DVE lanes
                    # run in parallel; [1,N] and [4,N] cost the same)
                    rc32 = spool.tile([128, 512], F32, tag="rc32",
                                      bufs=2, name="rc32")
                    with nc.allow_low_precision(
                            reason="1/den in fp16: 0.05% rel, den>=13"):
                        nc.vector.reciprocal(rc32[0:97, :], d4[0:97, :])
                    rc16 = spool.tile([128, 512], F16, tag="rc16", bufs=2,
                                      name="rc16")
                    nc.vector.tensor_copy(rc16[0:97, :], rc32[0:97, :])
                    den_st["rc"]["rc"] = rc16
                    den_st["tile"] = None

        def do_step(p, qo, qn):
            if True:
                flush_norm(2)
                prev = avq.pop(0) if avq else None
                if prev:
                    ptp_a, p_a, qo_a, qn_a = prev
                    avs2 = [pspool.tile([65, 512], F32, tag="avs", bufs=2,
                                        name="avs") for _ in range(2)]
                ptp = ppool.tile([128, kt, 2, 512], F16, tag="ptp",
                                 bufs=(4 if cap == CAP else 2),
                                 name=f"ptp{p}{qo}")
                for k in range(kt):
                    # both heads' dots adjacent at row groups 0/64 -> the
                    # PE runs them concurrently; one exp covers both.
                    # Previous step's AV matmuls are interleaved per-k so
                    # the PE has ready work while exp paces the dots.
                    dp = pspool.tile([128, 2, 512], F32, tag="dp", bufs=2)
                    for h01 in range(2):
                        rb = h01 * 64
                        nc.tensor.matmul(
                            dp[:, h01, 0:qn],
                            qkT["k"][rb:rb + 64, p,
                                     k * 128:(k + 1) * 128],
                            qkT["q"][rb:rb + 64, p, qo:qo + qn],
                            start=True, stop=True,
                            tile_position=(rb, 0))
                    nc.scalar.activation(
                        ptp[:, k, :, 0:qn], dp[:, :, 0:qn],
                        EXP, scale=0.125, bias=padb_sb[:, 0:1])
                    # diagonal zeroing when this kt's window is in-chunk
                    if qo <= k * 128 < qo + qn:
                        off = k * 128 - qo
                        w = min(qn - off, 128)
                        for h01 in range(2):
                            nc.vector.tensor_mul(
                                ptp[:, k, h01, off:off + w],
                                ptp[:, k, h01, off:off + w],
                                eye_sb[:, 0:w])
                    if prev:
                        for h01 in range(2):
                            nc.tensor.matmul(
                                avs2[h01][:, 0:qn_a],
                                vp[:, k, 2 * p_a + h01, :],
                                ptp_a[:, k, h01, 0:qn_a],
                                start=(k == 0), stop=(k == kt - 1),
                                perf_mode=DR)
                if prev:
                    finish_av(avs2, p_a, qo_a, qn_a)
                avq.append((ptp, p, qo, qn))

        # interleave attention steps with the remaining projection groups:
        # steps 0-1 need only groups 0-1, so the PE FIFO alternates proj
        # bursts with exp-paced attention instead of serializing them
        steps = [(p, qo, qn) for (qo, qn) in qch for p in range(4)]
        emit_vproj(range(0, 4))
        do_step(*steps[0])
        emit_vproj(range(4, kt))
        emit_qk_group(1)
        do_step(*steps[1])
        emit_qk_group(2)
        do_step(*steps[2])
        emit_qk_group(3)
        for st in steps[3:]:
            do_step(*st)

        def finish_partial_rc():
            if den_st["tile"] is not None and den_st["row"] > 0:
                d4 = den_st["tile"]
                rc32 = spool.tile([128, 512], F32, tag="rc32",
                                  bufs=2, name="rc32")
                with nc.allow_low_precision(
                        reason="1/den in fp16: 0.05% rel, den>=13"):
                    nc.vector.reciprocal(rc32[0:97, :], d4[0:97, :])
                rc16 = spool.tile([128, 512], F16, bufs=2, tag="rc16",
                                  name="rc16")
                nc.vector.tensor_copy(rc16[0:97, :], rc32[0:97, :])
                den_st["rc"]["rc"] = rc16
                den_st["tile"] = None

        # tail: AV + normalize for the final step
        ptp_a, p_a, qo_a, qn_a = avq.pop(0)
        avs2 = [pspool.tile([65, 512], F32, tag="avs", bufs=2, name="avs")
                for _ in range(2)]
        for k in range(kt):
            for h01 in range(2):
                nc.tensor.matmul(
                    avs2[h01][:, 0:qn_a],
                    vp[:, k, 2 * p_a + h01, :],
                    ptp_a[:, k, h01, 0:qn_a],
                    start=(k == 0), stop=(k == kt - 1),
                    perf_mode=DR)
        finish_av(avs2, p_a, qo_a, qn_a)
        finish_partial_rc()
        flush_norm(len(deferred))

        # defensive: emit any output chunk not already flushed out
        for qo_d, _ in qch:
            if normed.get(qo_d, 0) != 8 and qo_d in tt_of:
                raise AssertionError(f"chunk {qo_d} normed "
                                     f"{normed.get(qo_d, 0)} != 8")
    nc.finalize()
    return nc


_NC_CACHE = {}


def _get_nc(cap, fused_pairs):
    key = (cap, fused_pairs)
    if key not in _NC_CACHE:
        _NC_CACHE[key] = build_bass(cap, fused_pairs)
    return _NC_CACHE[key]


def kernel(encoder_outputs, mask, Wq, bq, Wk, bk, Wv, bv, Wc):
    global LAST_RESULTS
    x = np.asarray(encoder_outputs, dtype=np.float32)
    mask = np.asarray(mask)
    f16 = np.float16
    Wqh, Wkh, Wvh = (np.asarray(w, np.float32) for w in (Wq, Wk, Wv))
    Wch = np.asarray(Wc, np.float32)

    idxs = [np.where(mask[b] == 0)[0] for b in range(B)]
    nbs = [len(i) for i in idxs]
    if max(nbs) <= 1024 + 32 and min(nbs) >= 896:
        cap, fused = CAP, 3    # attention q-range trimmed to 1056
    else:
        cap, fused = S, 0          # generic fallback: no compaction gain
    kt = cap // 128

    eyebar = (1.0 - np.eye(128, dtype=np.float32)).astype(f16)
    onesv = np.ones((128, 64), dtype=f16)

    in_maps = []
    for c in range(NCORES):
        b, half = c // 2, c % 2
        hsl = slice(half * PD, (half + 1) * PD)
        idx, nb = idxs[b], nbs[b]
        xc = np.zeros((cap, H), np.float32)
        xc[:nb] = x[b, idx[:nb]]
        xcT = np.ascontiguousarray(xc.T).astype(f16)
        padb = np.full(cap, -1.0, dtype=np.float32)
        inv = (np.arange(cap) < nb).astype(np.float32)
        bqk = np.stack([np.asarray(bq, np.float32)[hsl].reshape(4, 128),
                        np.asarray(bk, np.float32)[hsl].reshape(4, 128)]
                       ).reshape(8, 128).T.copy()
        in_maps.append({
            "xcT": xcT,
            "wq": np.ascontiguousarray(Wqh[hsl, :].T).astype(f16),
            "wk": np.ascontiguousarray(Wkh[hsl, :].T).astype(f16),
            "wv": np.ascontiguousarray(Wvh[hsl, :].T).astype(f16),
            "wc": np.ascontiguousarray(Wch[:, hsl].T).astype(f16),
            "bqk": bqk,
            "padb": np.ascontiguousarray(padb.reshape(kt, 128).T),
            "eyebar": eyebar,
            "inv16": np.ascontiguousarray(
                inv.reshape(kt, 128).T.astype(np.float32)),
            "ones64": onesv,
        })

    res = run_bass_kernel_spmd(
        _get_nc(cap, fused), in_maps, list(range(NCORES)),
        trace=bool(os.environ.get("BASS_TRACE")))
    LAST_RESULTS = res

    bvWc = (np.asarray(bv, np.float64) @ np.asarray(Wc, np.float64).T
            ).astype(np.float32)
    out = np.zeros((B, S, H), dtype=np.float32)
    for b in range(B):
        nb = nbs[b]
        part = (res.results[2 * b]["out"][:nb].astype(np.float32)
                + res.results[2 * b + 1]["out"][:nb].astype(np.float32))
        out[b, idxs[b][:nb]] = part + bvWc[None, :]
    return out
